# revision 22
# baseline (speedup 1.0000x reference)
"""CapsuleLayer dynamic-routing kernel for 8 Trainium2 NeuronCores.

I-sharding: each core owns 144 of the 1152 input capsules.

Design (~564 us vs 1407 us baseline):
  - All matmuls bf16 (fp32 LOW_HIGH mode is 4x slower per row).
  - hat stored [b, i, (d,n)] (d-major, n innermost) in bf16: every routing
    multiply has an innermost-contiguous 16-bit access pattern (DVE 2x
    mode) and both reductions (over d and over i) are contiguous-halving
    in-place add trees (2x) instead of TensorReduce (1x, no fast mode).
  - s0 = (1/N) sum_i hat computed by a dedicated K=128 GEMM over (i,k)
    so its AllReduce (which pays ~80 us of ncfw first-collective latency)
    overlaps the whole hat-creation phase; routing starts as soon as it
    lands and chases creation block-by-block.
  - Routing entirely on DVE: GpSimd tensor ops poison DVE throughput via
    SBUF contention (measured: identical DVE instructions run at 1x
    instead of 2x while GpSimd streams), so GpSimd only runs DMA queues
    and collectives. PSUM drains split 40/60 DVE/ACT; exp on ACT.
  - bb logits and s accumulators bf16 (error gate is 2e-2; measured 6e-3).
Cross-core: AllReduce of s [128,512] bf16, 3x (first hidden under create).
Every core computes the identical final output; core 0's is returned.
"""

import os
import numpy as np
import ml_dtypes

import concourse.bass as bass
import concourse.bacc as bacc
import concourse.tile as tile
import concourse.mybir as mybir
from concourse import bass_utils

B, I, DIN = 128, 1152, 8
N, D = 32, 16
ND = N * D  # 512, flattened (d, n): nd = d*32 + n
NCORES = 8
IL = I // NCORES  # 144
EPS = 1e-7
F32 = mybir.dt.float32
BF16 = mybir.dt.bfloat16
CH = 2            # i-chunk for X/W streaming in create

# routing block split: DVE gets 7 blocks of 16 i, GpSimd 4 blocks of 8 i
DVE_BI, DVE_NBLK = 16, 9     # all i on DVE
GP_BI, GP_NBLK = 8, 0        # GP disabled (SBUF contention test)
GP_I0 = DVE_BI * DVE_NBLK


def _ap(ap: bass.AP, dims) -> bass.AP:
    """Rebuild `ap` with explicit free [step,count] dims (partition dim kept)."""
    return bass.AP(tensor=ap.tensor, offset=ap.offset, ap=[ap.ap[0]] + list(dims))


def build_nc():
    nc = bacc.Bacc(
        "TRN2",
        target_bir_lowering=False,
        debug=False,
        enable_asserts=True,
        num_devices=NCORES,
    )
    x_d = nc.dram_tensor("x", [DIN, IL, B], BF16, kind="ExternalInput").ap()
    w_d = nc.dram_tensor("w", [DIN, IL, ND], BF16, kind="ExternalInput").ap()
    x2_d = nc.dram_tensor("x2", [IL * DIN, B], BF16, kind="ExternalInput").ap()
    w2_d = nc.dram_tensor("w2", [IL * DIN, ND], BF16, kind="ExternalInput").ap()
    out_d = nc.dram_tensor("out", [B, ND], F32, kind="ExternalOutput").ap()

    AX = mybir.AxisListType.X
    ACT_COPY = mybir.ActivationFunctionType.Copy
    ACT_EXP = mybir.ActivationFunctionType.Exp
    ACT_SQRT = mybir.ActivationFunctionType.Sqrt

    with tile.TileContext(nc) as tc:
        with (
            tc.tile_pool(name="big", bufs=1) as big,
            tc.tile_pool(name="stream", bufs=2) as stream,
            tc.tile_pool(name="workbig", bufs=1) as workbig,
            tc.tile_pool(name="worksm", bufs=2) as worksm,
            tc.tile_pool(name="ps", bufs=5, space="PSUM") as pspool,
            tc.tile_pool(name="ps0", bufs=1, space="PSUM") as ps0pool,
            tc.tile_pool(name="dram", bufs=1, space="DRAM") as dram,
        ):
            hat = big.tile([B, IL, ND], BF16)           # 147.5 KB/part
            bb_dve = big.tile([B, GP_I0, N], BF16)      # 7 KB
            bb_gp = (big.tile([B, IL - GP_I0, N], BF16)
                     if GP_NBLK else None)              # 2 KB
            outT = big.tile([B, ND], F32)               # 2 KB
            tsq = outT                                  # alias: tsq dead before outT write
            outbf_t = big.tile([B, 1, ND], BF16)        # 1 KB
            sacc = big.tile([B, 1, ND], BF16)           # 1 KB s accumulator
            s_ar = big.tile([B, ND], BF16)              # 1 KB AR result
            smalls = big.tile([B, 5, N], F32)           # 0.6 KB
            s2, a1, r1, rt = (smalls[:, j, :] for j in range(4))
            eps_t = smalls[:, 4, 0:1]
            # persistent per-engine work tiles (single-buffered: same-engine
            # program order already serializes reuse)
            dprods = [workbig.tile([B, DVE_BI, ND], BF16, name=f"dprod{j}",
                                   tag=f"p{j}")
                      for j in range(2)]                 # 2 x 16 KB

            nc.vector.memset(eps_t, EPS)

            # ---------- s0 GEMM:  s0[b, nd] = sum_(i,k) x2 * w2  ----------
            s0ps = ps0pool.tile([B, ND], F32)
            NK = IL * DIN // B  # 9 chunks of K=128
            for j in range(NK):
                x2c = stream.tile([B, B], BF16, tag="x2")
                w2c = stream.tile([B, ND], BF16, tag="w2")
                nc.gpsimd.dma_start(out=x2c[:], in_=x2_d[j * B:(j + 1) * B, :])
                nc.sync.dma_start(out=w2c[:], in_=w2_d[j * B:(j + 1) * B, :])
                nc.tensor.matmul(
                    s0ps[:], lhsT=x2c[:], rhs=w2c[:],
                    start=(j == 0), stop=(j == NK - 1),
                )
            # s0 * (1/N) -> bf16 AR payload (into sacc, the AR staging source)
            nc.scalar.activation(out=sacc[:, 0, :], in_=s0ps[:], func=ACT_COPY,
                                 scale=1.0 / N)

            def allreduce_s():
                ar_in = dram.tile([B, ND], BF16, tag="arin")
                ar_out = dram.tile([B, ND], BF16, tag="arout")
                nc.gpsimd.dma_start(out=ar_in[:], in_=sacc[:, 0, :])
                nc.gpsimd.collective_compute(
                    "AllReduce",
                    mybir.AluOpType.add,
                    replica_groups=[list(range(NCORES))],
                    ins=[ar_in.opt()],
                    outs=[ar_out.opt()],
                )
                nc.gpsimd.dma_start(out=s_ar[:], in_=ar_out[:])

            allreduce_s()  # AR1 (s0) overlaps the create loop below

            # ---------- create hat ----------
            for ic in range(IL // CH):
                wt = stream.tile([DIN, CH, ND], BF16, tag="wch")
                xch = stream.tile([DIN, CH, B], BF16, tag="xch")
                nc.sync.dma_start(out=wt[:], in_=w_d[:, ic * CH:(ic + 1) * CH, :])
                nc.gpsimd.dma_start(out=xch[:], in_=x_d[:, ic * CH:(ic + 1) * CH, :])
                for j in range(CH):
                    i = ic * CH + j
                    ps = pspool.tile([B, ND], F32)
                    nc.tensor.matmul(
                        ps[:], lhsT=xch[:, j, :], rhs=wt[:, j, :],
                        start=True, stop=True,
                    )
                    if i % 2 == 0:
                        nc.vector.tensor_copy(hat[:, i, :], ps[:])
                    else:
                        nc.scalar.copy(out=hat[:, i, :], in_=ps[:])

            # ---------- squash: out from s (bf16) ----------
            def squash(r):
                last = (r == 2)
                s_in = s_ar[:]
                nc.vector.tensor_mul(tsq[:], s_in, s_in)     # f32 = bf16^2
                # s2[b,n] = sum_d tsq   (d: stride 32, count 16, innermost)
                nc.vector.reduce_sum(
                    out=s2, in_=_ap(tsq[:], [[1, N], [N, D]]), axis=AX)
                nc.vector.tensor_scalar_add(a1, s2, 1.0)
                nc.vector.reciprocal(out=r1, in_=a1)
                nc.vector.tensor_mul(r1, r1, s2)              # s2/(1+s2)
                nc.scalar.activation(out=rt, in_=s2, func=ACT_SQRT,
                                     bias=eps_t, scale=1.0)
                nc.vector.reciprocal(out=rt, in_=rt)
                nc.vector.tensor_mul(r1, r1, rt)              # scale [B,N]
                if last:
                    # outT[b, n, d] = s[b, d, n] * r1[b, n]
                    nc.vector.tensor_mul(
                        _ap(outT[:], [[D, N], [1, D]]),
                        _ap(s_in, [[1, N], [N, D]]),
                        _ap(r1, [[1, N], [0, D]]))
                    return None
                ob = outbf_t[:, 0, :]
                nc.vector.tensor_mul(
                    ob, s_in, _ap(r1, [[0, D], [1, N]]))      # bcast over d
                return ob

            # ---------- routing ----------
            def route_block(eng, prod, bbs, ee_tag, rr_tag, cee_tag,
                            BIb, hat_blk, ob, it, first_blk, sslot):
                # -- b-pass: prod = hat * out (bcast over i) --
                eng.tensor_mul(prod[:], hat_blk, _ap(ob, [[0, BIb], [1, ND]]))
                # in-place d-tree: contiguous halving over nd slabs
                for half in (256, 128, 64):
                    eng.tensor_add(prod[:, :, 0:half], prod[:, :, 0:half],
                                   prod[:, :, half:2 * half])
                if it == 0:
                    # final level writes bb directly
                    eng.tensor_add(bbs, prod[:, :, 0:N], prod[:, :, N:2 * N])
                else:
                    eng.tensor_add(prod[:, :, 0:N], prod[:, :, 0:N],
                                   prod[:, :, N:2 * N])
                    eng.tensor_add(bbs, bbs, prod[:, :, 0:N])
                # -- softmax over n (local per (b,i)) --
                ee = worksm.tile([B, BIb, N], BF16, tag=ee_tag)
                rr = worksm.tile([B, BIb], F32, tag=rr_tag)
                nc.scalar.activation(out=ee[:], in_=bbs, func=ACT_EXP,
                                     bias=eps_t, scale=1.0)
                nc.vector.reduce_sum(out=rr[:], in_=ee[:], axis=AX)
                nc.vector.reciprocal(out=rr[:], in_=rr[:])
                cee = worksm.tile([B, BIb, N], BF16, tag=cee_tag)
                eng.tensor_mul(cee[:], ee[:], _ap(rr[:], [[1, BIb], [0, N]]))
                # -- s-pass: prod = hat * c (bcast over d) --
                eng.tensor_mul(prod[:], hat_blk,
                               _ap(cee[:], [[N, BIb], [0, D], [1, N]]))
                # in-place i-tree
                half = BIb // 2
                while half >= 2:
                    eng.tensor_add(prod[:, 0:half, :], prod[:, 0:half, :],
                                   prod[:, half:2 * half, :])
                    half //= 2
                ss = sacc[:, 0, :]
                if first_blk:
                    eng.tensor_add(ss, prod[:, 0, :], prod[:, 1, :])
                else:
                    eng.tensor_add(prod[:, 0, :], prod[:, 0, :], prod[:, 1, :])
                    eng.tensor_add(ss, ss, prod[:, 0, :])

            for it in range(2):
                ob = squash(it)
                for blk in range(DVE_NBLK):
                    i0 = blk * DVE_BI
                    route_block(nc.vector, dprods[blk % 2],
                                bb_dve[:, i0:i0 + DVE_BI, :],
                                "dee", "drr", "dcee", DVE_BI,
                                hat[:, i0:i0 + DVE_BI, :], ob, it, blk == 0, 0)
                allreduce_s()

            squash(2)
            nc.sync.dma_start(out=out_d[:], in_=outT[:])

    nc.compile()
    return nc


_NC_CACHE = None


def kernel(inputs: np.ndarray, W: np.ndarray) -> np.ndarray:
    global _NC_CACHE
    if _NC_CACHE is None:
        _NC_CACHE = build_nc()
    nc = _NC_CACHE

    inputs = np.ascontiguousarray(inputs, dtype=np.float32)
    W = np.ascontiguousarray(W, dtype=np.float32)
    bf = ml_dtypes.bfloat16
    in_maps = []
    for c in range(NCORES):
        sl = slice(c * IL, (c + 1) * IL)
        xs = inputs[:, sl, :]                     # [B, IL, 8]
        ws = W[:, sl, :, :]                       # [N, IL, D, 8]
        x_c = np.ascontiguousarray(xs.transpose(2, 1, 0).astype(bf))
        w_c = np.ascontiguousarray(
            ws.transpose(3, 1, 2, 0).astype(bf)).reshape(DIN, IL, ND)
        x2_c = np.ascontiguousarray(
            xs.transpose(1, 2, 0).astype(bf)).reshape(IL * DIN, B)
        w2_c = np.ascontiguousarray(
            ws.transpose(1, 3, 2, 0).astype(bf)).reshape(IL * DIN, ND)
        in_maps.append({"x": x_c, "w": w_c, "x2": x2_c, "w2": w2_c})

    trace = bool(int(os.environ.get("CAPS_TRACE", "0")))
    res = bass_utils.run_bass_kernel_spmd(
        nc, in_maps, core_ids=list(range(NCORES)), trace=trace)
    if trace and res.exec_time_ns is not None:
        print(f"HW exec time: {res.exec_time_ns} ns")
    return res.results[0]["out"].reshape(B, N, D).astype(np.float32)


# revision 23
# speedup vs baseline: 1.1003x; 1.1003x over previous
"""CapsuleLayer dynamic-routing kernel for 8 Trainium2 NeuronCores.

I-sharding: each core owns 144 of the 1152 input capsules.

Design (~564 us vs 1407 us baseline):
  - All matmuls bf16 (fp32 LOW_HIGH mode is 4x slower per row).
  - hat stored [b, i, (d,n)] (d-major, n innermost) in bf16: every routing
    multiply has an innermost-contiguous 16-bit access pattern (DVE 2x
    mode) and both reductions (over d and over i) are contiguous-halving
    in-place add trees (2x) instead of TensorReduce (1x, no fast mode).
  - s0 = (1/N) sum_i hat computed by a dedicated K=128 GEMM over (i,k)
    so its AllReduce (which pays ~80 us of ncfw first-collective latency)
    overlaps the whole hat-creation phase; routing starts as soon as it
    lands and chases creation block-by-block.
  - Routing entirely on DVE: GpSimd tensor ops poison DVE throughput via
    SBUF contention (measured: identical DVE instructions run at 1x
    instead of 2x while GpSimd streams), so GpSimd only runs DMA queues
    and collectives. PSUM drains split 40/60 DVE/ACT; exp on ACT.
  - bb logits and s accumulators bf16 (error gate is 2e-2; measured 6e-3).
Cross-core: AllReduce of s [128,512] bf16, 3x (first hidden under create).
Every core computes the identical final output; core 0's is returned.
"""

import os
import numpy as np
import ml_dtypes

import concourse.bass as bass
import concourse.bacc as bacc
import concourse.tile as tile
import concourse.mybir as mybir
from concourse import bass_utils

B, I, DIN = 128, 1152, 8
N, D = 32, 16
ND = N * D  # 512, flattened (d, n): nd = d*32 + n
NCORES = 8
IL = I // NCORES  # 144
EPS = 1e-7
F32 = mybir.dt.float32
BF16 = mybir.dt.bfloat16
CH = 2            # i-chunk for X/W streaming in create

# routing block split: DVE gets 7 blocks of 16 i, GpSimd 4 blocks of 8 i
DVE_BI, DVE_NBLK = 16, 9     # all i on DVE
GP_BI, GP_NBLK = 8, 0        # GP disabled (SBUF contention test)
GP_I0 = DVE_BI * DVE_NBLK


def _ap(ap: bass.AP, dims) -> bass.AP:
    """Rebuild `ap` with explicit free [step,count] dims (partition dim kept)."""
    return bass.AP(tensor=ap.tensor, offset=ap.offset, ap=[ap.ap[0]] + list(dims))


def build_nc():
    nc = bacc.Bacc(
        "TRN2",
        target_bir_lowering=False,
        debug=False,
        enable_asserts=True,
        num_devices=NCORES,
    )
    x_d = nc.dram_tensor("x", [DIN, IL, B], BF16, kind="ExternalInput").ap()
    w_d = nc.dram_tensor("w", [DIN, IL, ND], BF16, kind="ExternalInput").ap()
    x2_d = nc.dram_tensor("x2", [IL * DIN, B], BF16, kind="ExternalInput").ap()
    w2_d = nc.dram_tensor("w2", [IL * DIN, ND], BF16, kind="ExternalInput").ap()
    out_d = nc.dram_tensor("out", [B, ND], F32, kind="ExternalOutput").ap()

    AX = mybir.AxisListType.X
    ACT_COPY = mybir.ActivationFunctionType.Copy
    ACT_EXP = mybir.ActivationFunctionType.Exp
    ACT_SQRT = mybir.ActivationFunctionType.Sqrt

    with tile.TileContext(nc) as tc:
        with (
            tc.tile_pool(name="big", bufs=1) as big,
            tc.tile_pool(name="stream", bufs=2) as stream,
            tc.tile_pool(name="workbig", bufs=1) as workbig,
            tc.tile_pool(name="worksm", bufs=2) as worksm,
            tc.tile_pool(name="ps", bufs=5, space="PSUM") as pspool,
            tc.tile_pool(name="ps0", bufs=1, space="PSUM") as ps0pool,
            tc.tile_pool(name="dram", bufs=1, space="DRAM") as dram,
        ):
            hat = big.tile([B, IL, ND], BF16)           # 147.5 KB/part
            bb_dve = big.tile([B, GP_I0, N], BF16)      # 7 KB
            bb_gp = (big.tile([B, IL - GP_I0, N], BF16)
                     if GP_NBLK else None)              # 2 KB
            outT = big.tile([B, ND], F32)               # 2 KB
            tsq = outT                                  # alias: tsq dead before outT write
            outbf_t = big.tile([B, 1, ND], BF16)        # 1 KB
            sacc = big.tile([B, 1, ND], BF16)           # 1 KB s accumulator
            s_ar = big.tile([B, ND], BF16)              # 1 KB AR result
            smalls = big.tile([B, 5, N], F32)           # 0.6 KB
            s2, a1, r1, rt = (smalls[:, j, :] for j in range(4))
            eps_t = smalls[:, 4, 0:1]
            # persistent per-engine work tiles (single-buffered: same-engine
            # program order already serializes reuse)
            dprods = [workbig.tile([B, DVE_BI, ND], BF16, name=f"dprod{j}",
                                   tag=f"p{j}")
                      for j in range(2)]                 # 2 x 16 KB

            nc.vector.memset(eps_t, EPS)

            # ---------- s0 GEMM:  s0[b, nd] = sum_(i,k) x2 * w2  ----------
            s0ps = ps0pool.tile([B, ND], F32)
            NK = IL * DIN // B  # 9 chunks of K=128
            for j in range(NK):
                x2c = stream.tile([B, B], BF16, tag="x2")
                w2c = stream.tile([B, ND], BF16, tag="w2")
                nc.gpsimd.dma_start(out=x2c[:], in_=x2_d[j * B:(j + 1) * B, :])
                nc.sync.dma_start(out=w2c[:], in_=w2_d[j * B:(j + 1) * B, :])
                nc.tensor.matmul(
                    s0ps[:], lhsT=x2c[:], rhs=w2c[:],
                    start=(j == 0), stop=(j == NK - 1),
                )
            # s0 * (1/N) -> bf16 AR payload (into sacc, the AR staging source)
            nc.scalar.activation(out=sacc[:, 0, :], in_=s0ps[:], func=ACT_COPY,
                                 scale=1.0 / N)

            def allreduce_s():
                ar_in = dram.tile([B, ND], BF16, tag="arin")
                ar_out = dram.tile([B, ND], BF16, tag="arout")
                nc.gpsimd.dma_start(out=ar_in[:], in_=sacc[:, 0, :])
                nc.gpsimd.collective_compute(
                    "AllReduce",
                    mybir.AluOpType.add,
                    replica_groups=[list(range(NCORES))],
                    ins=[ar_in.opt()],
                    outs=[ar_out.opt()],
                )
                nc.gpsimd.dma_start(out=s_ar[:], in_=ar_out[:])

            allreduce_s()  # AR1 (s0) overlaps the create loop below

            # ---------- create hat ----------
            for ic in range(IL // CH):
                wt = stream.tile([DIN, CH, ND], BF16, tag="wch")
                xch = stream.tile([DIN, CH, B], BF16, tag="xch")
                nc.sync.dma_start(out=wt[:], in_=w_d[:, ic * CH:(ic + 1) * CH, :])
                nc.gpsimd.dma_start(out=xch[:], in_=x_d[:, ic * CH:(ic + 1) * CH, :])
                for j in range(CH):
                    i = ic * CH + j
                    ps = pspool.tile([B, ND], F32)
                    nc.tensor.matmul(
                        ps[:], lhsT=xch[:, j, :], rhs=wt[:, j, :],
                        start=True, stop=True,
                    )
                    nc.scalar.copy(out=hat[:, i, :], in_=ps[:])

            # ---------- squash: out from s (bf16) ----------
            def squash(r):
                last = (r == 2)
                s_in = s_ar[:]
                nc.vector.tensor_mul(tsq[:], s_in, s_in)     # f32 = bf16^2
                # s2[b,n] = sum_d tsq   (d: stride 32, count 16, innermost)
                nc.vector.reduce_sum(
                    out=s2, in_=_ap(tsq[:], [[1, N], [N, D]]), axis=AX)
                nc.vector.tensor_scalar_add(a1, s2, 1.0)
                nc.vector.reciprocal(out=r1, in_=a1)
                nc.vector.tensor_mul(r1, r1, s2)              # s2/(1+s2)
                nc.scalar.activation(out=rt, in_=s2, func=ACT_SQRT,
                                     bias=eps_t, scale=1.0)
                nc.vector.reciprocal(out=rt, in_=rt)
                nc.vector.tensor_mul(r1, r1, rt)              # scale [B,N]
                if last:
                    # outT[b, n, d] = s[b, d, n] * r1[b, n]
                    nc.vector.tensor_mul(
                        _ap(outT[:], [[D, N], [1, D]]),
                        _ap(s_in, [[1, N], [N, D]]),
                        _ap(r1, [[1, N], [0, D]]))
                    return None
                ob = outbf_t[:, 0, :]
                nc.vector.tensor_mul(
                    ob, s_in, _ap(r1, [[0, D], [1, N]]))      # bcast over d
                return ob

            # ---------- routing ----------
            def route_block(eng, prod, bbs, ee_tag, rr_tag, cee_tag,
                            BIb, hat_blk, ob, it, first_blk, sslot):
                # -- b-pass: prod = hat * out (bcast over i) --
                eng.tensor_mul(prod[:], hat_blk, _ap(ob, [[0, BIb], [1, ND]]))
                # in-place d-tree: contiguous halving over nd slabs
                for half in (256, 128, 64):
                    eng.tensor_add(prod[:, :, 0:half], prod[:, :, 0:half],
                                   prod[:, :, half:2 * half])
                if it == 0:
                    # final level writes bb directly
                    eng.tensor_add(bbs, prod[:, :, 0:N], prod[:, :, N:2 * N])
                else:
                    eng.tensor_add(prod[:, :, 0:N], prod[:, :, 0:N],
                                   prod[:, :, N:2 * N])
                    eng.tensor_add(bbs, bbs, prod[:, :, 0:N])
                # -- softmax over n (local per (b,i)) --
                ee = worksm.tile([B, BIb, N], BF16, tag=ee_tag)
                rr = worksm.tile([B, BIb], F32, tag=rr_tag)
                nc.scalar.activation(out=ee[:], in_=bbs, func=ACT_EXP,
                                     bias=eps_t, scale=1.0)
                nc.vector.reduce_sum(out=rr[:], in_=ee[:], axis=AX)
                nc.vector.reciprocal(out=rr[:], in_=rr[:])
                cee = worksm.tile([B, BIb, N], BF16, tag=cee_tag)
                eng.tensor_mul(cee[:], ee[:], _ap(rr[:], [[1, BIb], [0, N]]))
                # -- s-pass: prod = hat * c (bcast over d) --
                eng.tensor_mul(prod[:], hat_blk,
                               _ap(cee[:], [[N, BIb], [0, D], [1, N]]))
                # in-place i-tree
                half = BIb // 2
                while half >= 2:
                    eng.tensor_add(prod[:, 0:half, :], prod[:, 0:half, :],
                                   prod[:, half:2 * half, :])
                    half //= 2
                ss = sacc[:, 0, :]
                if first_blk:
                    eng.tensor_add(ss, prod[:, 0, :], prod[:, 1, :])
                else:
                    eng.tensor_add(prod[:, 0, :], prod[:, 0, :], prod[:, 1, :])
                    eng.tensor_add(ss, ss, prod[:, 0, :])

            for it in range(2):
                ob = squash(it)
                for blk in range(DVE_NBLK):
                    i0 = blk * DVE_BI
                    route_block(nc.vector, dprods[blk % 2],
                                bb_dve[:, i0:i0 + DVE_BI, :],
                                "dee", "drr", "dcee", DVE_BI,
                                hat[:, i0:i0 + DVE_BI, :], ob, it, blk == 0, 0)
                allreduce_s()

            squash(2)
            nc.sync.dma_start(out=out_d[:], in_=outT[:])

    nc.compile()
    return nc


_NC_CACHE = None


def kernel(inputs: np.ndarray, W: np.ndarray) -> np.ndarray:
    global _NC_CACHE
    if _NC_CACHE is None:
        _NC_CACHE = build_nc()
    nc = _NC_CACHE

    inputs = np.ascontiguousarray(inputs, dtype=np.float32)
    W = np.ascontiguousarray(W, dtype=np.float32)
    bf = ml_dtypes.bfloat16
    in_maps = []
    for c in range(NCORES):
        sl = slice(c * IL, (c + 1) * IL)
        xs = inputs[:, sl, :]                     # [B, IL, 8]
        ws = W[:, sl, :, :]                       # [N, IL, D, 8]
        x_c = np.ascontiguousarray(xs.transpose(2, 1, 0).astype(bf))
        w_c = np.ascontiguousarray(
            ws.transpose(3, 1, 2, 0).astype(bf)).reshape(DIN, IL, ND)
        x2_c = np.ascontiguousarray(
            xs.transpose(1, 2, 0).astype(bf)).reshape(IL * DIN, B)
        w2_c = np.ascontiguousarray(
            ws.transpose(1, 3, 2, 0).astype(bf)).reshape(IL * DIN, ND)
        in_maps.append({"x": x_c, "w": w_c, "x2": x2_c, "w2": w2_c})

    trace = bool(int(os.environ.get("CAPS_TRACE", "0")))
    res = bass_utils.run_bass_kernel_spmd(
        nc, in_maps, core_ids=list(range(NCORES)), trace=trace)
    if trace and res.exec_time_ns is not None:
        print(f"HW exec time: {res.exec_time_ns} ns")
    return res.results[0]["out"].reshape(B, N, D).astype(np.float32)
